# revision 28
# baseline (speedup 1.0000x reference)
"""Cross-attention kernel for Trainium2, SPMD over 8 NeuronCores.

Problem: B=2, LQ=1024, LK=10000, E=256, H=8 heads of D=32.
  q = queries @ Wq + bq ; k = bev @ Wk (+bk dropped) ; v = bev @ Wv
  out = softmax(q k^T) v  @ Wo + bo

Sharding: core c -> (batch b = c // 4, head-pair hp = c % 4).  Each core
computes attention for its 2 heads of its batch plus the partial output
projection through its 64 rows of Wo.  Host sums the 4 partials per batch
and adds bo.  No collectives.

Structure: single chunk-streamed pass.  xq/xk are shipped fp16 from the
host; per 512-kpos chunk, hardware DMA-transpose (XBAR) lands xk^T in
SBUF with zero engine cost, then kT (fp16) and v (bf16, natural [kpos,
vdim] layout) are projected and the chunk is immediately consumed by 8
attention groups (kt-tile x head, N=1024 covering all queries): energy
matmul (fp16, 2-way row-packed by head), exp split between the
Activation engine (spline exp -> bf16, with a compensating bias) and DVE
(1-instruction Schraudolph bit-trick exp: int16 = round(A*x + B) whose
bits ARE bf16(e^x), ~3% rel err), and attn@v 2-way column-packed by head
(h0 -> PSUM partitions 0-32, h1 -> 64-96 via PE column tiling).  The
ones column appended to v yields the softmax denominator in the same
accumulating matmul.  attn@v emission lags 2 groups behind energy so the
in-order PE never head-of-line blocks on an exp.

The PE HAM clock gate only counts plain-fp32 matmul activity; tiny fp32
"warm" matmuls keep it at 8/8 (2.4 GHz) without tripping the activity_1
power throttler (which full fp32 transposes previously did).

Numerics: bk is dropped (constant-per-query energy shift cancels in
softmax); bv is added after normalization; fp16 q/k carry ~1e-3 relative
error, v/attn weights are bf16; the ACT exp carries a small constant
bias so both exp flavors share the same mean multiplicative error, which
cancels in the softmax ratio.
"""
import sys

sys.path.insert(0, "/opt/trn_rl_repo")

import numpy as np

B, LQ, LK, E, H = 2, 1024, 10000, 256, 8
D = 32            # head dim
HPC = 2           # heads per core
DC = D * HPC      # 64 projected dims per core
LKP = 10240       # LK padded to a multiple of 512
NCH = LKP // 512  # 20 chunks
NKT = LKP // 128  # 80 k-tiles

# Schraudolph int16 exp:  bf16_bits(e^x) ~= round(x * 128/ln2 + (127*128 - C))
SCH_A = 128.0 / float(np.log(2.0))
SCH_C = 6.0
SCH_B = 127.0 * 128.0 - SCH_C
ACT_BIAS = 0.00738          # ln(mean Schraudolph/exp ratio) at C=6
# which of the 8 (kt,h) groups per chunk go to DVE (by group index mod 8)
DVE_PAIRS = frozenset({2, 5, 7})
WARM_N = 1                  # fp32 warm matmuls per group

DBG_SKIP_UNITS = False
DBG_SKIP_TAIL = False

_CACHE = {}


def _build():
    import concourse.bacc as bacc
    import concourse.tile as tile
    from concourse import mybir

    FP32 = mybir.dt.float32
    FP16 = mybir.dt.float16
    BF16 = mybir.dt.bfloat16
    I16 = mybir.dt.int16
    AF = mybir.ActivationFunctionType
    OP = mybir.AluOpType

    nc = bacc.Bacc("TRN2", target_bir_lowering=False)

    XQ = nc.dram_tensor("xq", [LQ, E], FP16, kind="ExternalInput")
    XK = nc.dram_tensor("xk", [LKP, E], FP16, kind="ExternalInput")
    WQ = nc.dram_tensor("wq", [E, DC], FP32, kind="ExternalInput")
    WK = nc.dram_tensor("wk", [E, DC], FP32, kind="ExternalInput")
    WV = nc.dram_tensor("wv", [E, DC], FP32, kind="ExternalInput")
    WO = nc.dram_tensor("wo", [DC, E], FP32, kind="ExternalInput")
    BQ = nc.dram_tensor("bq", [DC], FP32, kind="ExternalInput")
    BV = nc.dram_tensor("bv", [DC], FP32, kind="ExternalInput")
    IDT = nc.dram_tensor("ident", [128, 128], FP32, kind="ExternalInput")
    # partial output, transposed: rows = embed dim, cols = query position
    OUT = nc.dram_tensor("out_t", [E, LQ], FP32, kind="ExternalOutput")

    with tile.TileContext(nc) as tc:
        with (
            tc.tile_pool(name="singles", bufs=1) as sg,
            tc.tile_pool(name="aio", bufs=2) as aio,
            tc.tile_pool(name="xkt", bufs=3) as xktp,
            tc.tile_pool(name="ktp", bufs=2) as ktp,
            tc.tile_pool(name="vap", bufs=2) as vap,
            tc.tile_pool(name="sta", bufs=4) as stap,
            tc.tile_pool(name="std", bufs=3) as stdp,
            tc.tile_pool(name="stg", bufs=2, space="PSUM") as stg,     # 4 banks
            tc.tile_pool(name="kvp", bufs=1, space="PSUM") as kvp,     # 2 banks
            tc.tile_pool(name="avp", bufs=1, space="PSUM") as avp,     # 2 banks
        ):
            # ---- constants / weights ----
            ident = sg.tile([128, 128], FP32, tag="ident")
            nc.sync.dma_start(out=ident, in_=IDT[:, :])

            ones = sg.tile([128, 32], FP32, tag="ones")
            nc.vector.memset(ones, 1.0)

            abias = sg.tile([128, 1], FP32, tag="abias")
            nc.vector.memset(abias, ACT_BIAS)

            def load_round(dram_ap, shape, tag, dt):
                f = aio.tile(shape, FP32, tag="wstage", name="wst_" + tag)
                nc.sync.dma_start(out=f, in_=dram_ap)
                r = sg.tile(shape, dt, tag=tag, name=tag)
                nc.vector.tensor_copy(r, f)
                return r

            wq_h = load_round(WQ[:, :].rearrange("(c p) m -> p c m", p=128),
                              [128, 2, DC], "wq", FP16)
            wk_h = load_round(WK[:, :].rearrange("(c p) m -> p c m", p=128),
                              [128, 2, DC], "wk", FP16)
            wv_h = load_round(WV[:, :].rearrange("(c p) m -> p c m", p=128),
                              [128, 2, DC], "wv", FP16)
            # Wo rows: head h's 32 rows on partitions 64h..64h+32 (the lane
            # each head's attention output lives in)
            wo_f = aio.tile([128, E], FP32, tag="wstage", name="wst_wo")
            for h in range(2):
                nc.sync.dma_start(out=wo_f[64 * h:64 * h + 32, :],
                                  in_=WO[32 * h:32 * (h + 1), :])
            wo_r = sg.tile([128, E], mybir.dt.float32r, tag="wo")
            nc.vector.tensor_copy(wo_r[0:32, :], wo_f[0:32, :])
            nc.vector.tensor_copy(wo_r[64:96, :], wo_f[64:96, :])

            bq_sb = sg.tile([64, 1], FP32, tag="bq")
            nc.sync.dma_start(out=bq_sb, in_=BQ[:].rearrange("(p o) -> p o", o=1))
            # bv: head h's 32 bias values on partitions 64h..64h+32
            bv_sb = sg.tile([128, 1], FP32, tag="bv")
            for h in range(2):
                nc.sync.dma_start(
                    out=bv_sb[64 * h:64 * h + 32, :],
                    in_=BV[32 * h:32 * (h + 1)].rearrange("(p o) -> p o", o=1))

            # ---- stage q: DMA-transpose queries, project q^T ----
            # qT rows 0-31 h0, 32-63 h1, 64-127 duplicate (row strips 2,3)
            qT = sg.tile([128, LQ], FP16, tag="qT")
            xqT = sg.tile([128, 2 * LQ], FP16, tag="xqT")
            for e in range(2):
                nc.sync.dma_start_transpose(
                    xqT[:, e * LQ:(e + 1) * LQ],
                    XQ[:, e * 128:(e + 1) * 128])
            qp = stg.tile([128, 1024], FP32, tag="stg", name="qp")
            for qc in range(2):
                qs = slice(qc * 512, (qc + 1) * 512)
                for e in range(2):
                    nc.tensor.matmul(qp[0:64, qs], wq_h[:, e, :],
                                     xqT[:, e * LQ + qc * 512:
                                         e * LQ + (qc + 1) * 512],
                                     start=(e == 0), stop=(e == 1))
            nc.vector.tensor_scalar_add(qT[0:64, :], qp[0:64, :], bq_sb[:, 0:1])
            nc.sync.dma_start(out=qT[64:128, :], in_=qT[0:64, :])

            # ---- attention accumulator ----
            # av: partitions 0-31 h0 v-sums, 32 h0 denom,
            #     64-95 h1 v-sums, 96 h1 denom; cols = query position
            av = avp.tile([128, 1024], FP32, tag="av")

            pend = []
            n_grp = [0]
            kv_warm = [None]

            def warm(n):
                # Plain-fp32 mini-matmuls: the PE HAM activity monitor counts
                # only the fp32 path; a trickle of counted activity holds the
                # clock gate at 8/8 (2.4 GHz) for everything else.
                for _ in range(n):
                    nc.tensor.matmul(kv_warm[0][96:128, 448:480],
                                     ident[0:32, 0:32], ident[0:32, 0:32],
                                     start=True, stop=True,
                                     tile_position=(0, 96),
                                     skip_group_check=True)

            def emit_attnv(rec):
                kt, h, vaug, rhs = rec
                for qc in range(2):
                    qs = slice(qc * 512, (qc + 1) * 512)
                    nc.tensor.matmul(
                        av[h * 64:h * 64 + 33, qs],
                        vaug[:, (kt % 4 * 2 + h) * 33:
                             (kt % 4 * 2 + h) * 33 + 33],
                        rhs[:, qs], start=(kt == 0),
                        stop=(kt == NKT - 1),
                        tile_position=(0, h * 64), skip_group_check=True)

            def emit_group(kt, h, kT_c, vaug):
                g = n_grp[0]
                s = stg.tile([128, 1024], FP32, tag="stg", name=f"s{g}")
                for qc in range(2):
                    qs = slice(qc * 512, (qc + 1) * 512)
                    for m in range(2):
                        # M=64 halves: keeps fp16 weight loads off the FWL
                        # path (NumWeights==128 would auto-enable it)
                        nc.tensor.matmul(
                            s[m * 64:(m + 1) * 64, qs],
                            kT_c[h * 32:h * 32 + 32,
                                 (kt % 4) * 128 + m * 64:
                                 (kt % 4) * 128 + (m + 1) * 64],
                            qT[h * 32:h * 32 + 32, qs],
                            start=True, stop=True,
                            tile_position=(h * 32, m * 64),
                            skip_group_check=True)
                warm(WARM_N)
                if (g % 8) in DVE_PAIRS:
                    st = stdp.tile([128, 1024], I16, tag="sTd", name=f"sTd{g}")
                    nc.vector.tensor_scalar(st, s, SCH_A, SCH_B, OP.mult, OP.add)
                    rhs = st[:, :].bitcast(BF16)
                else:
                    st = stap.tile([128, 1024], BF16, tag="sTa", name=f"sTa{g}")
                    nc.scalar.activation(st, s, AF.Exp, bias=abias[:, 0:1])
                    rhs = st[:, :]
                pend.append((kt, h, vaug, rhs))
                if len(pend) > 2:
                    emit_attnv(pend.pop(0))
                n_grp[0] += 1

            # ---- chunk loop: stream K/V prep + attention groups ----
            for c in range(NCH):
                xkT = xktp.tile([128, 1024], FP16, tag="xkT", name=f"xkT{c}")
                for e in range(2):
                    nc.sync.dma_start_transpose(
                        xkT[:, e * 512:(e + 1) * 512],
                        XK[c * 512:(c + 1) * 512, e * 128:(e + 1) * 128])

                kv = kvp.tile([128, 512], FP32, tag="kv", name=f"kv{c}")
                kv_warm[0] = kv
                kp = kvp.tile([128, 512], FP32, tag="kp", name=f"kp{c}")
                for e in range(2):
                    nc.tensor.matmul(
                        kp[0:64, :], wk_h[:, e, :],
                        xkT[:, e * 512:(e + 1) * 512],
                        start=(e == 0), stop=(e == 1))
                kT_c = ktp.tile([64, 512], FP16, tag="kT", name=f"kT{c}")
                nc.vector.tensor_copy(kT_c, kp[0:64, :])

                # v natural [kpos, vdim]: per kt-tile, 2 accumulating e-halves
                for t in range(4):
                    for m in range(2):
                        for e in range(2):
                            nc.tensor.matmul(
                                kv[m * 64:(m + 1) * 64, t * 64:(t + 1) * 64],
                                xkT[:, e * 512 + t * 128 + m * 64:
                                    e * 512 + t * 128 + (m + 1) * 64],
                                wv_h[:, e, :],
                                start=(e == 0), stop=(e == 1),
                                skip_group_check=True)
                if c == 0:
                    warm(30)
                vaug = vap.tile([128, 264], BF16, tag="vaug", name=f"va{c}")
                nc.vector.tensor_copy(
                    vaug[:, :].rearrange("p (k a b) -> p k a b", a=2, b=33)
                    [:, :, :, 0:32],
                    kv[:, 0:256].rearrange("p (k a b) -> p k a b", a=2, b=32))
                nc.vector.tensor_copy(
                    vaug[:, :].rearrange("p (k b) -> p k b", b=33)[:, :, 32:33],
                    ones[:, 0:8].rearrange("p (k b) -> p k b", b=1))

                if not DBG_SKIP_UNITS:
                    for kt_in_chunk in range(4):
                        for h in range(2):
                            emit_group(c * 4 + kt_in_chunk, h, kT_c, vaug)
            while pend:
                emit_attnv(pend.pop(0))

            # ---- tail: normalize, bias, output projection ----
            out_sb = [sg.tile([128, LQ], FP32, tag=f"out{e}", name=f"out{e}")
                      for e in range(2)]
            # attnT: h0 rows on partitions 0-31, h1 on 64-95
            attnT = sg.tile([128, LQ], mybir.dt.float32r, tag="attnT")
            if not DBG_SKIP_TAIL:
                avs = sg.tile([128, 1024], FP32, tag="avs")
                nc.vector.tensor_copy(avs[0:33, :], av[0:33, :])
                nc.vector.tensor_copy(avs[64:97, :], av[64:97, :])
                rb = stg.tile([128, 1024], FP32, tag="stg", name="rb")
                for qc in range(2):
                    qs = slice(qc * 512, (qc + 1) * 512)
                    nc.tensor.matmul(rb[0:32, qs], ones[32:33, 0:32],
                                     avs[32:33, qs],
                                     start=True, stop=True,
                                     tile_position=(32, 0))
                    nc.tensor.matmul(rb[64:96, qs], ones[96:97, 0:32],
                                     avs[96:97, qs],
                                     start=True, stop=True,
                                     tile_position=(96, 64))
                rbs = sg.tile([128, 1024], FP32, tag="rbs")
                nc.vector.reciprocal(rbs[0:32, :], rb[0:32, :])
                nc.vector.reciprocal(rbs[64:96, :], rb[64:96, :])
                tmp = sg.tile([128, 1024], FP32, tag="ctmp")
                for h in range(2):
                    hp = slice(64 * h, 64 * h + 32)
                    nc.vector.tensor_mul(tmp[hp, :], avs[hp, :], rbs[hp, :])
                    nc.vector.tensor_scalar_add(attnT[hp, :], tmp[hp, :],
                                                bv_sb[hp, 0:1])
            else:
                for ec in range(2):
                    nc.vector.memset(out_sb[ec], 0.0)
            for qc in ([] if DBG_SKIP_TAIL else [0, 1]):
                qs = slice(qc * 512, (qc + 1) * 512)
                for ec in range(2):
                    po = [stg.tile([128, 1024], FP32, tag="stg",
                                   name=f"po{qc}{ec}{h}") for h in range(2)]
                    for h in range(2):
                        hp = slice(64 * h, 64 * h + 32)
                        nc.tensor.matmul(
                            po[h][:, 0:512], wo_r[hp, ec * 128:(ec + 1) * 128],
                            attnT[hp, qs], start=True, stop=True,
                            tile_position=(64 * h, 0), skip_group_check=True)
                    nc.scalar.activation(out_sb[ec][:, qs], po[0][:, 0:512],
                                         AF.Copy)
                    nc.vector.tensor_tensor(
                        out_sb[ec][:, qs], out_sb[ec][:, qs], po[1][:, 0:512],
                        OP.add)
            for ec in range(2):
                nc.sync.dma_start(out=OUT[ec * 128:(ec + 1) * 128, :],
                                  in_=out_sb[ec])

    nc.compile()
    return nc


def _get_nc():
    if "nc" not in _CACHE:
        _CACHE["nc"] = _build()
    return _CACHE["nc"]


def kernel(bev_emb, queries, Wq, bq, Wk, bk, Wv, bv, Wo, bo):
    from concourse.bass_utils import run_bass_kernel_spmd

    bev_emb = np.asarray(bev_emb, dtype=np.float32)
    queries = np.asarray(queries, dtype=np.float32)
    Wq = np.asarray(Wq, dtype=np.float32)
    bq = np.asarray(bq, dtype=np.float32)
    Wk = np.asarray(Wk, dtype=np.float32)
    Wv = np.asarray(Wv, dtype=np.float32)
    bv = np.asarray(bv, dtype=np.float32)
    Wo = np.asarray(Wo, dtype=np.float32)
    bo = np.asarray(bo, dtype=np.float32)

    xk_pad = np.zeros((B, LKP, E), dtype=np.float16)
    xk_pad[:, :LK, :] = bev_emb.astype(np.float16)
    xq16 = queries.astype(np.float16)
    ident = np.eye(128, dtype=np.float32)

    in_maps = []
    for c in range(8):
        b, hp = c // 4, c % 4
        hs = slice(hp * DC, (hp + 1) * DC)
        in_maps.append({
            "xq": np.ascontiguousarray(xq16[b]),
            "xk": np.ascontiguousarray(xk_pad[b]),
            "wq": np.ascontiguousarray(Wq[:, hs]),
            "wk": np.ascontiguousarray(Wk[:, hs]),
            "wv": np.ascontiguousarray(Wv[:, hs]),
            "wo": np.ascontiguousarray(Wo[hs, :]),
            "bq": np.ascontiguousarray(bq[hs]),
            "bv": np.ascontiguousarray(bv[hs]),
            "ident": ident,
        })

    nc = _get_nc()
    _CACHE["last_in_maps"] = in_maps
    res = run_bass_kernel_spmd(nc, in_maps, list(range(8)))
    _CACHE["last_result"] = res

    out = np.zeros((B, LQ, E), dtype=np.float32)
    for c in range(8):
        out[c // 4] += res.results[c]["out_t"].T
    out += bo
    return out
